# revision 21
# baseline (speedup 1.0000x reference)
"""Trainium2 Bass kernel for nn_MultiHeadAttention (B=4, S=2048, D=1024, H=16).

Sharding: 8 cores = 4 batches x 2 head-groups. Core c handles batch c//2,
heads [8*(c%2), 8*(c%2)+8). Host sums the two c_proj partials per batch.

v2 design (from trace analysis of the 503us baseline):
  - all matmul operands bf16 (FWL weight loads, half DMA bytes)
  - x^T resident in SBUF (loaded once, reused by all pairs' QK chains)
  - scores PSUM ring of 3 tiles [128,2h,512q] absorbs scheduling jitter so
    the ScalarE exp stream (the ~280us floor) never stalls
  - software-pipelined emission: scores lead AV by 4 steps so the PE queue
    never head-of-line blocks the exp stream
  - denominator: bf16 A/B accumulation chains on DVE; at block boundary a
    ones[128,64]-weighted matmul replicates sum-over-keys across all
    partitions (h0 -> psum rows 0:64, h1 -> 64:128, col-tiled), one DVE
    reciprocal + one tensor_mul normalizes avs -> ATN (no gpsimd bcast)
  - pair-outer / qc-inner(512) blocks; QK(p+1), V, and c_proj(qc) run as
    small PE filler quanta inside the block streams; c_proj overlaps the
    pair-3 blocks so the tail is one q-chunk
"""

import contextlib
import ctypes
import os
import sys
import types

import numpy as np
import ml_dtypes

# ---------------------------------------------------------------------------
# NTFF profiling hook (used when BASS_PROBLEM_TRACE=1)
# ---------------------------------------------------------------------------
_AXON_SO = "/opt/axon/libaxon_pjrt.so"


def _install_ntff_hook():
    if "antenv.axon_hooks" in sys.modules:
        return
    try:
        import antenv
    except ImportError:
        return
    try:
        lib = ctypes.CDLL(_AXON_SO)
    except OSError:
        return
    if not hasattr(lib, "axon_start_nrt_profile"):
        return
    lib.axon_start_nrt_profile.argtypes = [
        ctypes.POINTER(ctypes.c_int64),
        ctypes.c_size_t,
    ]
    lib.axon_start_nrt_profile.restype = ctypes.c_int64
    lib.axon_stop_nrt_profile.argtypes = [ctypes.c_char_p]
    lib.axon_stop_nrt_profile.restype = ctypes.c_int64

    @contextlib.contextmanager
    def _hook(output_dir, device_ids):
        import jax

        jax.devices()
        if device_ids:
            ids = (ctypes.c_int64 * len(device_ids))(*device_ids)
            rc = lib.axon_start_nrt_profile(ids, len(device_ids))
        else:
            rc = lib.axon_start_nrt_profile(None, 0)
        if rc != 0:
            raise RuntimeError(f"axon_start_nrt_profile rc={rc}")
        try:
            yield
        finally:
            n = lib.axon_stop_nrt_profile(str(output_dir).encode())
            print(f"profile: {n} file(s) written to {output_dir}", file=sys.stderr)

    mod = types.ModuleType("antenv.axon_hooks")
    holder = [_hook]
    mod.get_axon_ntff_profile_hook = lambda: holder[0]
    mod.set_axon_ntff_profile_hook = lambda h: holder.__setitem__(0, h)
    sys.modules["antenv.axon_hooks"] = mod
    antenv.axon_hooks = mod


_install_ntff_hook()

# ---------------------------------------------------------------------------
# Problem constants (hardcoded per the contract)
# ---------------------------------------------------------------------------
B, S, D = 4, 2048, 1024
H, DK = 16, 64
N_CORES = 8
HPC = 8            # heads per core
NPAIR = HPC // 2   # head pairs per core = 4
FC = HPC * DK      # features per core = 512
SCALE = 1.0 / float(np.sqrt(DK))  # 0.125

KC = D // 128      # 8 contraction chunks for qkv projections
NSC = 4            # seq chunks of 512 for x / QK tiles
KB = S // 128      # 16 key blocks
NQC = 4            # q chunks of 512
LAG = 4            # AV trails scores by this many kb steps

_CACHED = {}


def _build():
    import concourse.tile as tile
    from concourse import bacc, mybir

    f32 = mybir.dt.float32
    bf16 = mybir.dt.bfloat16
    Exp = mybir.ActivationFunctionType.Exp

    nc = bacc.Bacc("TRN2", target_bir_lowering=False, debug=False,
                   num_devices=N_CORES)

    # Pre-swizzled DRAM inputs (host packs these; contiguous per partition)
    xs = nc.dram_tensor("xs", [NSC, 128, KC, 512], bf16,
                        kind="ExternalInput").ap()
    wqk = nc.dram_tensor("wqk", [NPAIR, 128, 2, KC, 128], bf16,
                         kind="ExternalInput").ap()
    wv = nc.dram_tensor("wv", [128, KC, FC], bf16, kind="ExternalInput").ap()
    wp = nc.dram_tensor("wp", [128, NPAIR, D], bf16,
                        kind="ExternalInput").ap()
    out = nc.dram_tensor("out", [S, D], f32, kind="ExternalOutput").ap()
    dbg_on = os.environ.get("BASS_DEBUG_DUMP", "0") == "1"
    if dbg_on:
        dqt = nc.dram_tensor("dqt", [128, S], bf16, kind="ExternalOutput").ap()
        dkt = nc.dram_tensor("dkt", [128, S], bf16, kind="ExternalOutput").ap()
        dvt = nc.dram_tensor("dvt", [128, KB, FC], bf16,
                             kind="ExternalOutput").ap()
        datn = nc.dram_tensor("datn", [128, NPAIR, S], bf16,
                              kind="ExternalOutput").ap()
        dacc = nc.dram_tensor("dacc", [128, 2, 512], bf16,
                              kind="ExternalOutput").ap()

    with tile.TileContext(nc) as tc:
        with (
            tc.tile_pool(name="xsp", bufs=NSC) as xs_pool,
            tc.tile_pool(name="wqkp", bufs=NPAIR) as wqk_pool,
            tc.tile_pool(name="wvp", bufs=1) as wv_pool,
            tc.tile_pool(name="wpp", bufs=1) as wp_pool,
            tc.tile_pool(name="qkp", bufs=2) as qk_pool,
            tc.tile_pool(name="vtp", bufs=1) as v_pool,
            tc.tile_pool(name="ptp", bufs=24) as pt_pool,
            tc.tile_pool(name="accp", bufs=2) as acc_pool,
            tc.tile_pool(name="invp", bufs=2) as inv_pool,
            tc.tile_pool(name="atnp", bufs=1) as atn_pool,
            tc.tile_pool(name="outp", bufs=3) as out_pool,
            tc.tile_pool(name="cstp", bufs=1) as cst_pool,
            tc.tile_pool(name="stp", bufs=2, space="PSUM") as st_pool,
            tc.tile_pool(name="avp", bufs=2, space="PSUM") as av_pool,
            tc.tile_pool(name="shp", bufs=2, space="PSUM") as sh_pool,
        ):
            # ---------------- static tiles ----------------
            ones = cst_pool.tile([128, 64], bf16, tag="ones")
            nc.gpsimd.memset(ones[:], 1.0)

            XS = [xs_pool.tile([128, KC, 512], bf16, tag="xs",
                               name=f"xs_{sc}") for sc in range(NSC)]
            WV = wv_pool.tile([128, KC, FC], bf16, tag="wv")
            WP = wp_pool.tile([128, NPAIR, D], bf16, tag="wp")
            Vt = v_pool.tile([128, KB, FC], bf16, tag="vt")
            ATN = atn_pool.tile([128, NPAIR, S], bf16, tag="atn")

            WQK = []
            for p in range(NPAIR):
                t = wqk_pool.tile([128, 2, KC, 128], bf16, tag="wqk",
                                  name=f"wqk_{p}")
                WQK.append(t)

            # DMA order: pair0 weights + x chunk 0 first (prologue critical
            # path), then everything else.
            for t in (1, 0):
                for kc in range(0, KC, 2):
                    nc.sync.dma_start(WQK[0][:, t, kc:kc + 2],
                                      wqk[0, :, t, kc:kc + 2])
            for kc in range(KC):
                nc.sync.dma_start(XS[0][:, kc, :], xs[0, :, kc, :])
            for kc in range(0, KC, 2):
                nc.sync.dma_start(WV[:, kc:kc + 2, :], wv[:, kc:kc + 2, :])
            for sc in range(1, NSC):
                for kc in range(0, KC, 2):
                    nc.sync.dma_start(XS[sc][:, kc:kc + 2, :],
                                      xs[sc, :, kc:kc + 2, :])
            for p in range(1, NPAIR):
                nc.sync.dma_start(WQK[p][:], wqk[p])
            nc.sync.dma_start(WP[:], wp[:])

            # QT/KT tiles per pair (ring of 2)
            QK = {}

            def alloc_qk(p):
                qt = qk_pool.tile([128, S], bf16, tag="qt", name=f"qt_{p}")
                kt = qk_pool.tile([128, S], bf16, tag="kt", name=f"kt_{p}")
                QK[p] = (qt, kt)

            # ---------------- filler quanta ----------------
            uid = [0]

            def proj_quantum(dst_ap, w_ap, x_ap):
                """dst_ap [128,512]bf16 <- sum_kc w_ap[:,kc,:].T @ x_ap[:,kc,:]"""
                uid[0] += 1
                ps = sh_pool.tile([128, 512], f32, tag="sh",
                                  name=f"prj{uid[0]}")
                pslice = ps[:]
                first = None
                for kc in range(KC):
                    mm = nc.tensor.matmul(pslice, lhsT=w_ap[:, kc, :],
                                          rhs=x_ap[:, kc, :],
                                          start=(kc == 0),
                                          stop=(kc == KC - 1))
                    first = first or mm
                nc.vector.tensor_copy(dst_ap, pslice)
                return first

            def v_quantum(sc, sbl):
                kb = sc * 4 + sbl
                uid[0] += 1
                ps = sh_pool.tile([128, 512], f32, tag="sh", name=f"v{kb}")
                first = None
                for kc in range(KC):
                    mm = nc.tensor.matmul(
                        ps[:], lhsT=XS[sc][:, kc, sbl * 128:(sbl + 1) * 128],
                        rhs=WV[:, kc, :],
                        start=(kc == 0), stop=(kc == KC - 1))
                    first = first or mm
                nc.vector.tensor_copy(Vt[:, kb, :], ps[:])
                return first

            def qk_fillers(p):
                alloc_qk(p)
                qt, kt = QK[p]
                w = WQK[p]
                thunks = []
                for sc in range(NSC):
                    thunks.append(lambda sc=sc: proj_quantum(
                        kt[:, sc * 512:(sc + 1) * 512], w[:, 1], XS[sc]))
                for sc in range(NSC):
                    thunks.append(lambda sc=sc: proj_quantum(
                        qt[:, sc * 512:(sc + 1) * 512], w[:, 0], XS[sc]))
                return thunks

            def cproj_quantum(qc, qb, nn):
                ps = sh_pool.tile([128, 512], f32, tag="sh",
                                  name=f"cp{qc}_{qb}_{nn}")
                first = None
                for p in range(NPAIR):
                    mm = nc.tensor.matmul(
                        ps[:],
                        lhsT=ATN[:, p, qc * 512 + qb * 128:
                                 qc * 512 + (qb + 1) * 128],
                        rhs=WP[:, p, nn * 512:(nn + 1) * 512],
                        start=(p == 0), stop=(p == NPAIR - 1))
                    first = first or mm
                ot = out_pool.tile([128, 512], f32, tag="ot",
                                   name=f"o{qc}_{qb}_{nn}")
                nc.vector.tensor_copy(ot[:], ps[:])
                r0 = qc * 512 + qb * 128
                for hh in range(2):
                    nc.sync.dma_start(
                        out[r0:r0 + 128,
                            nn * 512 + hh * 256:nn * 512 + (hh + 1) * 256],
                        ot[:, hh * 256:(hh + 1) * 256])
                return first

            def cproj_fillers(qc):
                return [lambda qb=qb, nn=nn: cproj_quantum(qc, qb, nn)
                        for qb in range(4) for nn in range(2)]

            # ---------------- prologue ----------------
            # KT pair0 fully + QT pair0 chunk 0; alternate the psum target
            # between the shared bank and an st-ring slot so the chains
            # double-buffer against their DVE casts. V rides as the first
            # block's fillers (one V block per kb step, AV lags by LAG).
            alloc_qk(0)
            qt0, kt0 = QK[0]
            chains = [
                (kt0[:, 0:512], WQK[0][:, 1], XS[0]),
                (qt0[:, 0:512], WQK[0][:, 0], XS[0]),
                (qt0[:, 512:1024], WQK[0][:, 0], XS[1]),
            ]
            for dst, w, x in chains:
                proj_quantum(dst, w, x)

            # ---------------- per-pair filler queues ----------------
            # Emission deadlines (Tile deps follow emission order!):
            #   V(kb)   before av_step(kb) of block (p0,qc0)  -> slot kb
            #   qt0[sc] before scores of block (p0,qc=sc)
            #   QK(p+1) fully before pair p+1's first block
            #   cproj(qc) only after boundary(3,qc) was emitted
            late_kt0 = [
                lambda sc=sc: proj_quantum(
                    kt0[:, sc * 512:(sc + 1) * 512], WQK[0][:, 1], XS[sc])
                for sc in range(1, NSC)]
            late_qt0 = [
                lambda sc=sc: proj_quantum(
                    qt0[:, sc * 512:(sc + 1) * 512], WQK[0][:, 0], XS[sc])
                for sc in range(2, NSC)]
            v_all = [lambda sc=sc, sbl=sbl: v_quantum(sc, sbl)
                     for sc in range(NSC) for sbl in range(4)]
            # Queue order = emission deadlines: KT chunk j before scores
            # kb=4j; V(kb) at slot kb+3 (av pops at kb+4); remaining QT
            # chunks before their q-chunk's block; then pair-1's QK.
            filler_q = {
                0: late_kt0 + v_all + late_qt0 + qk_fillers(1),
                1: qk_fillers(2),
                2: qk_fillers(3),
                3: [],
            }
            # filler cadence per pair: p0 every step (V deadline), p1/p2
            # sparse (spread PE load), p3 every other step (8 cproj quanta)
            cadence = {0: 1, 1: 4, 2: 4, 3: 2}

            # ---------------- boundary ----------------
            def emit_boundary(pend):
                p, qc, avs, accA = pend
                dpsb = sh_pool.tile([128, 512], f32, tag="sh",
                                    name=f"dps_{p}_{qc}")
                for h in range(2):
                    osl = dpsb[h * 64:(h + 1) * 64, :]
                    nc.tensor.matmul(osl, lhsT=ones[:], rhs=accA[:, h, :],
                                     start=True, stop=True,
                                     tile_position=(0, h * 64),
                                     skip_group_check=True)
                inv = inv_pool.tile([128, 512], f32, tag="inv",
                                    name=f"inv_{p}_{qc}")
                nc.vector.reciprocal_approx_fast(inv[:], dpsb[:])
                nc.vector.tensor_mul(
                    ATN[:, p, qc * 512:(qc + 1) * 512], avs[:], inv[:])

            # ---------------- main attention blocks ----------------
            # Scores/exp form one continuous high-priority stream; AV steps
            # trail globally (across block boundaries) via a deque so late V
            # blocks or boundary work can never stall the exp stream. Each
            # block's boundary is emitted right after its last AV.
            av_pending = []  # (av_fn, boundary_tuple_or_None)

            def pump_avs(min_keep):
                while len(av_pending) > min_keep:
                    av_fn, bnd = av_pending.pop(0)
                    av_fn()
                    if bnd is not None:
                        emit_boundary(bnd)
                        if bnd[0] == 3 and bnd[1] < NQC - 1:
                            filler_q[3].extend(cproj_fillers(bnd[1]))

            gstep = [0]
            last_exp = [None]

            def _dep(inst):
                return getattr(inst, "ins", inst)
            for p in range(NPAIR):
                QTp, KTp = QK[p]
                fq = filler_q[p]
                fi = [0]

                def filler_step(fq=fq, fi=fi):
                    if fi[0] < len(fq):
                        # Scheduling-only dependency: the quantum's first MM
                        # may not be scheduled before the latest exp, so the
                        # frozen per-engine order interleaves chains with the
                        # exp stream instead of front-running it.
                        first = fq[fi[0]]()
                        if first is not None and last_exp[0] is not None:
                            tile.add_dep_helper(
                                _dep(first), _dep(last_exp[0]),
                                reason="filler paced by exp stream")
                        fi[0] += 1

                for qc in range(NQC):
                    qsl = slice(qc * 512, (qc + 1) * 512)
                    avs = av_pool.tile([128, 512], f32, tag="avs",
                                       name=f"avs_{p}_{qc}")
                    accA = acc_pool.tile([128, 2, 512], bf16, tag="accA",
                                         name=f"accA_{p}_{qc}")
                    pts = {}

                    def scores_step(kb, p=p, qc=qc, QTp=QTp, KTp=KTp,
                                    accA=accA, pts=pts, qsl=qsl):
                        st = st_pool.tile([128, 2, 512], f32, tag="st",
                                          name=f"st_{p}_{qc}_{kb}")
                        ksl = slice(kb * 128, (kb + 1) * 128)
                        pt = pt_pool.tile([128, 2, 512], bf16, tag="pt",
                                          name=f"pt_{p}_{qc}_{kb}")
                        with tc.high_priority(offset=100000):
                            for h in range(2):
                                hsl = slice(64 * h, 64 * h + 64)
                                nc.tensor.matmul(st[:, h, :],
                                                 lhsT=KTp[hsl, ksl],
                                                 rhs=QTp[hsl, qsl],
                                                 start=True, stop=True)
                            last_exp[0] = nc.scalar.activation(
                                pt[:], st[:], Exp, scale=SCALE)
                        pts[kb] = pt
                        if kb == 0:
                            nc.vector.tensor_copy(accA[:], pt[:])
                        else:
                            nc.vector.tensor_add(accA[:], accA[:], pt[:])

                    def av_step(kb, p=p, avs=avs, pts=pts):
                        pt = pts.pop(kb)
                        for h in range(2):
                            nc.tensor.matmul(
                                avs[64 * h:64 * h + 64, :],
                                lhsT=Vt[:, kb, p * 128 + 64 * h:
                                        p * 128 + 64 * h + 64],
                                rhs=pt[:, h, :],
                                start=(kb == 0), stop=(kb == KB - 1),
                                tile_position=(0, 64 * h),
                                skip_group_check=True)

                    cad = cadence[p]
                    for kb in range(KB):
                        scores_step(kb)
                        gstep[0] += 1
                        bnd = (p, qc, avs, accA) if kb == KB - 1 else None
                        av_pending.append(
                            (lambda kb=kb, f=av_step: f(kb), bnd))
                        pump_avs(LAG)
                        if kb % cad == 0:
                            filler_step()
                    filler_step()

                # drain remaining fillers before the next pair needs them
                while fi[0] < len(fq):
                    fq[fi[0]]()
                    fi[0] += 1

            # tail: drain AVs (emits the last boundary) + final c_proj
            lastacc = av_pending[-1][1][3]
            pump_avs(0)
            for q in cproj_fillers(NQC - 1):
                q()
            if dbg_on:
                qtd, ktd = QK[0]
                nc.sync.dma_start(dqt[:, :], qtd[:, :])
                nc.sync.dma_start(dkt[:, :], ktd[:, :])
                nc.sync.dma_start(dvt[:, :, :], Vt[:, :, :])
                nc.sync.dma_start(datn[:, :, :], ATN[:, :, :])
                nc.sync.dma_start(dacc[:, :, :], lastacc[:, :, :])

    nc.compile()
    return nc


def _get_nc():
    if "nc" not in _CACHED:
        _CACHED["nc"] = _build()
    return _CACHED["nc"]


def _shard(x, W_attn, W_proj):
    """Build per-core input maps with pre-swizzled bf16 layouts."""
    bf = ml_dtypes.bfloat16
    x = np.asarray(x, dtype=np.float32)
    W_attn = np.asarray(W_attn, dtype=np.float32)
    W_proj = np.asarray(W_proj, dtype=np.float32)
    in_maps = []
    for c in range(N_CORES):
        b, g = c // 2, c % 2
        # xs[sc, part, kc, j] = x[b, sc*512+j, kc*128+part]
        xt = x[b].T                                  # [D, S]
        xs_ = xt.reshape(KC, 128, NSC, 512).transpose(2, 1, 0, 3)
        # wqk[p, part, t, kc, f] = W_attn[kc*128+part, t*D + g*FC + p*128+f]
        # (partition dim second so the [128,2,KC,128] SBUF tile DMA is a
        # straight linear copy)
        wqk_ = np.empty((NPAIR, 128, 2, KC, 128), dtype=bf)
        for t in range(2):
            wslab = W_attn[:, t * D + g * FC: t * D + (g + 1) * FC]  # [D,FC]
            wr = wslab.reshape(KC, 128, NPAIR, 128).transpose(2, 1, 0, 3)
            wqk_[:, :, t] = wr.astype(bf)
        wv_ = W_attn[:, 2 * D + g * FC: 2 * D + (g + 1) * FC]        # [D,FC]
        wv_ = wv_.reshape(KC, 128, FC).transpose(1, 0, 2).astype(bf)
        # wp[part, p, m] = W_proj[g*FC + p*128 + part, m]
        wp_ = W_proj[g * FC:(g + 1) * FC, :].reshape(NPAIR, 128, D)
        wp_ = wp_.transpose(1, 0, 2).astype(bf)
        in_maps.append({
            "xs": np.ascontiguousarray(xs_.astype(bf)),
            "wqk": np.ascontiguousarray(wqk_),
            "wv": np.ascontiguousarray(wv_),
            "wp": np.ascontiguousarray(wp_),
        })
    return in_maps


def kernel(x, W_attn, W_proj):
    from concourse.bass_utils import run_bass_kernel_spmd

    nc = _get_nc()
    in_maps = _shard(x, W_attn, W_proj)
    trace = os.environ.get("BASS_PROBLEM_TRACE", "0") == "1"
    res = run_bass_kernel_spmd(nc, in_maps, list(range(N_CORES)), trace=trace)
    _CACHED["last_result"] = res
    out = np.empty((B, S, D), dtype=np.float32)
    for b in range(B):
        out[b] = res.results[2 * b]["out"] + res.results[2 * b + 1]["out"]
    return out


# revision 22
# speedup vs baseline: 1.0197x; 1.0197x over previous
"""Trainium2 Bass kernel for nn_MultiHeadAttention (B=4, S=2048, D=1024, H=16).

Sharding: 8 cores = 4 batches x 2 head-groups. Core c handles batch c//2,
heads [8*(c%2), 8*(c%2)+8). Host sums the two c_proj partials per batch.

v2 design (from trace analysis of the 503us baseline):
  - all matmul operands bf16 (FWL weight loads, half DMA bytes)
  - x^T resident in SBUF (loaded once, reused by all pairs' QK chains)
  - scores PSUM ring of 3 tiles [128,2h,512q] absorbs scheduling jitter so
    the ScalarE exp stream (the ~280us floor) never stalls
  - software-pipelined emission: scores lead AV by 4 steps so the PE queue
    never head-of-line blocks the exp stream
  - denominator: bf16 A/B accumulation chains on DVE; at block boundary a
    ones[128,64]-weighted matmul replicates sum-over-keys across all
    partitions (h0 -> psum rows 0:64, h1 -> 64:128, col-tiled), one DVE
    reciprocal + one tensor_mul normalizes avs -> ATN (no gpsimd bcast)
  - pair-outer / qc-inner(512) blocks; QK(p+1), V, and c_proj(qc) run as
    small PE filler quanta inside the block streams; c_proj overlaps the
    pair-3 blocks so the tail is one q-chunk
"""

import contextlib
import ctypes
import os
import sys
import types

import numpy as np
import ml_dtypes

# ---------------------------------------------------------------------------
# NTFF profiling hook (used when BASS_PROBLEM_TRACE=1)
# ---------------------------------------------------------------------------
_AXON_SO = "/opt/axon/libaxon_pjrt.so"


def _install_ntff_hook():
    if "antenv.axon_hooks" in sys.modules:
        return
    try:
        import antenv
    except ImportError:
        return
    try:
        lib = ctypes.CDLL(_AXON_SO)
    except OSError:
        return
    if not hasattr(lib, "axon_start_nrt_profile"):
        return
    lib.axon_start_nrt_profile.argtypes = [
        ctypes.POINTER(ctypes.c_int64),
        ctypes.c_size_t,
    ]
    lib.axon_start_nrt_profile.restype = ctypes.c_int64
    lib.axon_stop_nrt_profile.argtypes = [ctypes.c_char_p]
    lib.axon_stop_nrt_profile.restype = ctypes.c_int64

    @contextlib.contextmanager
    def _hook(output_dir, device_ids):
        import jax

        jax.devices()
        if device_ids:
            ids = (ctypes.c_int64 * len(device_ids))(*device_ids)
            rc = lib.axon_start_nrt_profile(ids, len(device_ids))
        else:
            rc = lib.axon_start_nrt_profile(None, 0)
        if rc != 0:
            raise RuntimeError(f"axon_start_nrt_profile rc={rc}")
        try:
            yield
        finally:
            n = lib.axon_stop_nrt_profile(str(output_dir).encode())
            print(f"profile: {n} file(s) written to {output_dir}", file=sys.stderr)

    mod = types.ModuleType("antenv.axon_hooks")
    holder = [_hook]
    mod.get_axon_ntff_profile_hook = lambda: holder[0]
    mod.set_axon_ntff_profile_hook = lambda h: holder.__setitem__(0, h)
    sys.modules["antenv.axon_hooks"] = mod
    antenv.axon_hooks = mod


_install_ntff_hook()

# ---------------------------------------------------------------------------
# Problem constants (hardcoded per the contract)
# ---------------------------------------------------------------------------
B, S, D = 4, 2048, 1024
H, DK = 16, 64
N_CORES = 8
HPC = 8            # heads per core
NPAIR = HPC // 2   # head pairs per core = 4
FC = HPC * DK      # features per core = 512
SCALE = 1.0 / float(np.sqrt(DK))  # 0.125

KC = D // 128      # 8 contraction chunks for qkv projections
NSC = 4            # seq chunks of 512 for x / QK tiles
KB = S // 128      # 16 key blocks
NQC = 4            # q chunks of 512
LAG = 4            # AV trails scores by this many kb steps

_CACHED = {}


def _build():
    import concourse.tile as tile
    from concourse import bacc, mybir

    f32 = mybir.dt.float32
    bf16 = mybir.dt.bfloat16
    Exp = mybir.ActivationFunctionType.Exp

    nc = bacc.Bacc("TRN2", target_bir_lowering=False, debug=False,
                   num_devices=N_CORES)

    # Pre-swizzled DRAM inputs (host packs these; contiguous per partition)
    xs = nc.dram_tensor("xs", [NSC, 128, KC, 512], bf16,
                        kind="ExternalInput").ap()
    wqk = nc.dram_tensor("wqk", [NPAIR, 128, 2, KC, 128], bf16,
                         kind="ExternalInput").ap()
    wv = nc.dram_tensor("wv", [128, KC, FC], bf16, kind="ExternalInput").ap()
    wp = nc.dram_tensor("wp", [128, NPAIR, D], bf16,
                        kind="ExternalInput").ap()
    out = nc.dram_tensor("out", [S, D], f32, kind="ExternalOutput").ap()
    dbg_on = os.environ.get("BASS_DEBUG_DUMP", "0") == "1"
    if dbg_on:
        dqt = nc.dram_tensor("dqt", [128, S], bf16, kind="ExternalOutput").ap()
        dkt = nc.dram_tensor("dkt", [128, S], bf16, kind="ExternalOutput").ap()
        dvt = nc.dram_tensor("dvt", [128, KB, FC], bf16,
                             kind="ExternalOutput").ap()
        datn = nc.dram_tensor("datn", [128, NPAIR, S], bf16,
                              kind="ExternalOutput").ap()
        dacc = nc.dram_tensor("dacc", [128, 2, 512], bf16,
                              kind="ExternalOutput").ap()

    with tile.TileContext(nc) as tc:
        with (
            tc.tile_pool(name="xsp", bufs=NSC) as xs_pool,
            tc.tile_pool(name="wqkp", bufs=NPAIR) as wqk_pool,
            tc.tile_pool(name="wvp", bufs=1) as wv_pool,
            tc.tile_pool(name="wpp", bufs=1) as wp_pool,
            tc.tile_pool(name="qkp", bufs=2) as qk_pool,
            tc.tile_pool(name="vtp", bufs=1) as v_pool,
            tc.tile_pool(name="ptp", bufs=24) as pt_pool,
            tc.tile_pool(name="accp", bufs=2) as acc_pool,
            tc.tile_pool(name="invp", bufs=2) as inv_pool,
            tc.tile_pool(name="atnp", bufs=1) as atn_pool,
            tc.tile_pool(name="outp", bufs=3) as out_pool,
            tc.tile_pool(name="cstp", bufs=1) as cst_pool,
            tc.tile_pool(name="stp", bufs=2, space="PSUM") as st_pool,
            tc.tile_pool(name="avp", bufs=2, space="PSUM") as av_pool,
            tc.tile_pool(name="shp", bufs=2, space="PSUM") as sh_pool,
        ):
            # ---------------- static tiles ----------------
            ones = cst_pool.tile([128, 64], bf16, tag="ones")
            nc.gpsimd.memset(ones[:], 1.0)

            XS = [xs_pool.tile([128, KC, 512], bf16, tag="xs",
                               name=f"xs_{sc}") for sc in range(NSC)]
            WV = wv_pool.tile([128, KC, FC], bf16, tag="wv")
            WP = wp_pool.tile([128, NPAIR, D], bf16, tag="wp")
            Vt = v_pool.tile([128, KB, FC], bf16, tag="vt")
            ATN = atn_pool.tile([128, NPAIR, S], bf16, tag="atn")

            WQK = []
            for p in range(NPAIR):
                t = wqk_pool.tile([128, 2, KC, 128], bf16, tag="wqk",
                                  name=f"wqk_{p}")
                WQK.append(t)

            # DMA order: pair0 weights + x chunk 0 first (prologue critical
            # path), then everything else.
            for t in (1, 0):
                nc.sync.dma_start(WQK[0][:, t], wqk[0, :, t])
            for kc in range(KC):
                nc.sync.dma_start(XS[0][:, kc, :], xs[0, :, kc, :])
            for kc in range(0, KC, 2):
                nc.sync.dma_start(WV[:, kc:kc + 2, :], wv[:, kc:kc + 2, :])
            for sc in range(1, NSC):
                for kc in range(0, KC, 2):
                    nc.sync.dma_start(XS[sc][:, kc:kc + 2, :],
                                      xs[sc, :, kc:kc + 2, :])
            for p in range(1, NPAIR):
                nc.sync.dma_start(WQK[p][:], wqk[p])
            nc.sync.dma_start(WP[:], wp[:])

            # QT/KT tiles per pair (ring of 2)
            QK = {}

            def alloc_qk(p):
                qt = qk_pool.tile([128, S], bf16, tag="qt", name=f"qt_{p}")
                kt = qk_pool.tile([128, S], bf16, tag="kt", name=f"kt_{p}")
                QK[p] = (qt, kt)

            # ---------------- filler quanta ----------------
            uid = [0]

            def proj_quantum(dst_ap, w_ap, x_ap):
                """dst_ap [128,512]bf16 <- sum_kc w_ap[:,kc,:].T @ x_ap[:,kc,:]"""
                uid[0] += 1
                ps = sh_pool.tile([128, 512], f32, tag="sh",
                                  name=f"prj{uid[0]}")
                pslice = ps[:]
                first = None
                for kc in range(KC):
                    mm = nc.tensor.matmul(pslice, lhsT=w_ap[:, kc, :],
                                          rhs=x_ap[:, kc, :],
                                          start=(kc == 0),
                                          stop=(kc == KC - 1))
                    first = first or mm
                nc.vector.tensor_copy(dst_ap, pslice)
                return first

            def v_quantum(sc, sbl):
                kb = sc * 4 + sbl
                uid[0] += 1
                ps = sh_pool.tile([128, 512], f32, tag="sh", name=f"v{kb}")
                first = None
                for kc in range(KC):
                    mm = nc.tensor.matmul(
                        ps[:], lhsT=XS[sc][:, kc, sbl * 128:(sbl + 1) * 128],
                        rhs=WV[:, kc, :],
                        start=(kc == 0), stop=(kc == KC - 1))
                    first = first or mm
                nc.vector.tensor_copy(Vt[:, kb, :], ps[:])
                return first

            def qk_fillers(p):
                alloc_qk(p)
                qt, kt = QK[p]
                w = WQK[p]
                thunks = []
                for sc in range(NSC):
                    thunks.append(lambda sc=sc: proj_quantum(
                        kt[:, sc * 512:(sc + 1) * 512], w[:, 1], XS[sc]))
                for sc in range(NSC):
                    thunks.append(lambda sc=sc: proj_quantum(
                        qt[:, sc * 512:(sc + 1) * 512], w[:, 0], XS[sc]))
                return thunks

            def cproj_quantum(qc, qb, nn):
                ps = sh_pool.tile([128, 512], f32, tag="sh",
                                  name=f"cp{qc}_{qb}_{nn}")
                first = None
                for p in range(NPAIR):
                    mm = nc.tensor.matmul(
                        ps[:],
                        lhsT=ATN[:, p, qc * 512 + qb * 128:
                                 qc * 512 + (qb + 1) * 128],
                        rhs=WP[:, p, nn * 512:(nn + 1) * 512],
                        start=(p == 0), stop=(p == NPAIR - 1))
                    first = first or mm
                ot = out_pool.tile([128, 512], f32, tag="ot",
                                   name=f"o{qc}_{qb}_{nn}")
                nc.vector.tensor_copy(ot[:], ps[:])
                r0 = qc * 512 + qb * 128
                nc.sync.dma_start(
                    out[r0:r0 + 128, nn * 512:(nn + 1) * 512], ot[:])
                return first

            def cproj_fillers(qc):
                return [lambda qb=qb, nn=nn: cproj_quantum(qc, qb, nn)
                        for qb in range(4) for nn in range(2)]

            # ---------------- prologue ----------------
            # KT pair0 fully + QT pair0 chunk 0; alternate the psum target
            # between the shared bank and an st-ring slot so the chains
            # double-buffer against their DVE casts. V rides as the first
            # block's fillers (one V block per kb step, AV lags by LAG).
            alloc_qk(0)
            qt0, kt0 = QK[0]
            chains = [
                (kt0[:, 0:512], WQK[0][:, 1], XS[0]),
                (qt0[:, 0:512], WQK[0][:, 0], XS[0]),
                (qt0[:, 512:1024], WQK[0][:, 0], XS[1]),
            ]
            for dst, w, x in chains:
                proj_quantum(dst, w, x)

            # ---------------- per-pair filler queues ----------------
            # Emission deadlines (Tile deps follow emission order!):
            #   V(kb)   before av_step(kb) of block (p0,qc0)  -> slot kb
            #   qt0[sc] before scores of block (p0,qc=sc)
            #   QK(p+1) fully before pair p+1's first block
            #   cproj(qc) only after boundary(3,qc) was emitted
            late_kt0 = [
                lambda sc=sc: proj_quantum(
                    kt0[:, sc * 512:(sc + 1) * 512], WQK[0][:, 1], XS[sc])
                for sc in range(1, NSC)]
            late_qt0 = [
                lambda sc=sc: proj_quantum(
                    qt0[:, sc * 512:(sc + 1) * 512], WQK[0][:, 0], XS[sc])
                for sc in range(2, NSC)]
            v_all = [lambda sc=sc, sbl=sbl: v_quantum(sc, sbl)
                     for sc in range(NSC) for sbl in range(4)]
            # Queue order = emission deadlines: KT chunk j before scores
            # kb=4j; V(kb) at slot kb+3 (av pops at kb+4); remaining QT
            # chunks before their q-chunk's block; then pair-1's QK.
            filler_q = {
                0: late_kt0 + v_all + late_qt0 + qk_fillers(1),
                1: qk_fillers(2),
                2: qk_fillers(3),
                3: [],
            }
            # filler cadence per pair: p0 every step (V deadline), p1/p2
            # sparse (spread PE load), p3 every other step (8 cproj quanta)
            cadence = {0: 1, 1: 4, 2: 4, 3: 2}

            # ---------------- boundary ----------------
            def emit_boundary(pend):
                p, qc, avs, accA = pend
                dpsb = sh_pool.tile([128, 512], f32, tag="sh",
                                    name=f"dps_{p}_{qc}")
                for h in range(2):
                    osl = dpsb[h * 64:(h + 1) * 64, :]
                    nc.tensor.matmul(osl, lhsT=ones[:], rhs=accA[:, h, :],
                                     start=True, stop=True,
                                     tile_position=(0, h * 64),
                                     skip_group_check=True)
                inv = inv_pool.tile([128, 512], f32, tag="inv",
                                    name=f"inv_{p}_{qc}")
                nc.vector.reciprocal_approx_fast(inv[:], dpsb[:])
                nc.vector.tensor_mul(
                    ATN[:, p, qc * 512:(qc + 1) * 512], avs[:], inv[:])

            # ---------------- main attention blocks ----------------
            # Scores/exp form one continuous high-priority stream; AV steps
            # trail globally (across block boundaries) via a deque so late V
            # blocks or boundary work can never stall the exp stream. Each
            # block's boundary is emitted right after its last AV.
            av_pending = []  # (av_fn, boundary_tuple_or_None)

            def pump_avs(min_keep):
                while len(av_pending) > min_keep:
                    av_fn, bnd = av_pending.pop(0)
                    av_fn()
                    if bnd is not None:
                        emit_boundary(bnd)
                        if bnd[0] == 3 and bnd[1] < NQC - 1:
                            filler_q[3].extend(cproj_fillers(bnd[1]))

            gstep = [0]
            last_exp = [None]

            def _dep(inst):
                return getattr(inst, "ins", inst)
            for p in range(NPAIR):
                QTp, KTp = QK[p]
                fq = filler_q[p]
                fi = [0]

                def filler_step(fq=fq, fi=fi):
                    if fi[0] < len(fq):
                        # Scheduling-only dependency: the quantum's first MM
                        # may not be scheduled before the latest exp, so the
                        # frozen per-engine order interleaves chains with the
                        # exp stream instead of front-running it.
                        first = fq[fi[0]]()
                        if first is not None and last_exp[0] is not None:
                            tile.add_dep_helper(
                                _dep(first), _dep(last_exp[0]),
                                reason="filler paced by exp stream")
                        fi[0] += 1

                for qc in range(NQC):
                    qsl = slice(qc * 512, (qc + 1) * 512)
                    avs = av_pool.tile([128, 512], f32, tag="avs",
                                       name=f"avs_{p}_{qc}")
                    accA = acc_pool.tile([128, 2, 512], bf16, tag="accA",
                                         name=f"accA_{p}_{qc}")
                    pts = {}

                    def scores_step(kb, p=p, qc=qc, QTp=QTp, KTp=KTp,
                                    accA=accA, pts=pts, qsl=qsl):
                        st = st_pool.tile([128, 2, 512], f32, tag="st",
                                          name=f"st_{p}_{qc}_{kb}")
                        ksl = slice(kb * 128, (kb + 1) * 128)
                        pt = pt_pool.tile([128, 2, 512], bf16, tag="pt",
                                          name=f"pt_{p}_{qc}_{kb}")
                        with tc.high_priority(offset=100000):
                            for h in range(2):
                                hsl = slice(64 * h, 64 * h + 64)
                                nc.tensor.matmul(st[:, h, :],
                                                 lhsT=KTp[hsl, ksl],
                                                 rhs=QTp[hsl, qsl],
                                                 start=True, stop=True)
                            last_exp[0] = nc.scalar.activation(
                                pt[:], st[:], Exp, scale=SCALE)
                        pts[kb] = pt
                        if kb == 0:
                            nc.vector.tensor_copy(accA[:], pt[:])
                        else:
                            nc.vector.tensor_add(accA[:], accA[:], pt[:])

                    def av_step(kb, p=p, avs=avs, pts=pts):
                        pt = pts.pop(kb)
                        for h in range(2):
                            nc.tensor.matmul(
                                avs[64 * h:64 * h + 64, :],
                                lhsT=Vt[:, kb, p * 128 + 64 * h:
                                        p * 128 + 64 * h + 64],
                                rhs=pt[:, h, :],
                                start=(kb == 0), stop=(kb == KB - 1),
                                tile_position=(0, 64 * h),
                                skip_group_check=True)

                    cad = cadence[p]
                    for kb in range(KB):
                        scores_step(kb)
                        gstep[0] += 1
                        bnd = (p, qc, avs, accA) if kb == KB - 1 else None
                        av_pending.append(
                            (lambda kb=kb, f=av_step: f(kb), bnd))
                        pump_avs(LAG)
                        if kb % cad == 0:
                            filler_step()
                    filler_step()

                # drain remaining fillers before the next pair needs them
                while fi[0] < len(fq):
                    fq[fi[0]]()
                    fi[0] += 1

            # tail: drain AVs (emits the last boundary) + final c_proj
            lastacc = av_pending[-1][1][3]
            pump_avs(0)
            for q in cproj_fillers(NQC - 1):
                q()
            if dbg_on:
                qtd, ktd = QK[0]
                nc.sync.dma_start(dqt[:, :], qtd[:, :])
                nc.sync.dma_start(dkt[:, :], ktd[:, :])
                nc.sync.dma_start(dvt[:, :, :], Vt[:, :, :])
                nc.sync.dma_start(datn[:, :, :], ATN[:, :, :])
                nc.sync.dma_start(dacc[:, :, :], lastacc[:, :, :])

    nc.compile()
    return nc


def _get_nc():
    if "nc" not in _CACHED:
        _CACHED["nc"] = _build()
    return _CACHED["nc"]


def _shard(x, W_attn, W_proj):
    """Build per-core input maps with pre-swizzled bf16 layouts."""
    bf = ml_dtypes.bfloat16
    x = np.asarray(x, dtype=np.float32)
    W_attn = np.asarray(W_attn, dtype=np.float32)
    W_proj = np.asarray(W_proj, dtype=np.float32)
    in_maps = []
    for c in range(N_CORES):
        b, g = c // 2, c % 2
        # xs[sc, part, kc, j] = x[b, sc*512+j, kc*128+part]
        xt = x[b].T                                  # [D, S]
        xs_ = xt.reshape(KC, 128, NSC, 512).transpose(2, 1, 0, 3)
        # wqk[p, part, t, kc, f] = W_attn[kc*128+part, t*D + g*FC + p*128+f]
        # (partition dim second so the [128,2,KC,128] SBUF tile DMA is a
        # straight linear copy)
        wqk_ = np.empty((NPAIR, 128, 2, KC, 128), dtype=bf)
        for t in range(2):
            wslab = W_attn[:, t * D + g * FC: t * D + (g + 1) * FC]  # [D,FC]
            wr = wslab.reshape(KC, 128, NPAIR, 128).transpose(2, 1, 0, 3)
            wqk_[:, :, t] = wr.astype(bf)
        wv_ = W_attn[:, 2 * D + g * FC: 2 * D + (g + 1) * FC]        # [D,FC]
        wv_ = wv_.reshape(KC, 128, FC).transpose(1, 0, 2).astype(bf)
        # wp[part, p, m] = W_proj[g*FC + p*128 + part, m]
        wp_ = W_proj[g * FC:(g + 1) * FC, :].reshape(NPAIR, 128, D)
        wp_ = wp_.transpose(1, 0, 2).astype(bf)
        in_maps.append({
            "xs": np.ascontiguousarray(xs_.astype(bf)),
            "wqk": np.ascontiguousarray(wqk_),
            "wv": np.ascontiguousarray(wv_),
            "wp": np.ascontiguousarray(wp_),
        })
    return in_maps


def kernel(x, W_attn, W_proj):
    from concourse.bass_utils import run_bass_kernel_spmd

    nc = _get_nc()
    in_maps = _shard(x, W_attn, W_proj)
    trace = os.environ.get("BASS_PROBLEM_TRACE", "0") == "1"
    res = run_bass_kernel_spmd(nc, in_maps, list(range(N_CORES)), trace=trace)
    _CACHED["last_result"] = res
    out = np.empty((B, S, D), dtype=np.float32)
    for b in range(B):
        out[b] = res.results[2 * b]["out"] + res.results[2 * b + 1]["out"]
    return out


# revision 23
# speedup vs baseline: 1.0288x; 1.0090x over previous
"""Trainium2 Bass kernel for nn_MultiHeadAttention (B=4, S=2048, D=1024, H=16).

Sharding: 8 cores = 4 batches x 2 head-groups. Core c handles batch c//2,
heads [8*(c%2), 8*(c%2)+8). Host sums the two c_proj partials per batch.

v2 design (from trace analysis of the 503us baseline):
  - all matmul operands bf16 (FWL weight loads, half DMA bytes)
  - x^T resident in SBUF (loaded once, reused by all pairs' QK chains)
  - scores PSUM ring of 3 tiles [128,2h,512q] absorbs scheduling jitter so
    the ScalarE exp stream (the ~280us floor) never stalls
  - software-pipelined emission: scores lead AV by 4 steps so the PE queue
    never head-of-line blocks the exp stream
  - denominator: bf16 A/B accumulation chains on DVE; at block boundary a
    ones[128,64]-weighted matmul replicates sum-over-keys across all
    partitions (h0 -> psum rows 0:64, h1 -> 64:128, col-tiled), one DVE
    reciprocal + one tensor_mul normalizes avs -> ATN (no gpsimd bcast)
  - pair-outer / qc-inner(512) blocks; QK(p+1), V, and c_proj(qc) run as
    small PE filler quanta inside the block streams; c_proj overlaps the
    pair-3 blocks so the tail is one q-chunk
"""

import contextlib
import ctypes
import os
import sys
import types

import numpy as np
import ml_dtypes

# ---------------------------------------------------------------------------
# NTFF profiling hook (used when BASS_PROBLEM_TRACE=1)
# ---------------------------------------------------------------------------
_AXON_SO = "/opt/axon/libaxon_pjrt.so"


def _install_ntff_hook():
    if "antenv.axon_hooks" in sys.modules:
        return
    try:
        import antenv
    except ImportError:
        return
    try:
        lib = ctypes.CDLL(_AXON_SO)
    except OSError:
        return
    if not hasattr(lib, "axon_start_nrt_profile"):
        return
    lib.axon_start_nrt_profile.argtypes = [
        ctypes.POINTER(ctypes.c_int64),
        ctypes.c_size_t,
    ]
    lib.axon_start_nrt_profile.restype = ctypes.c_int64
    lib.axon_stop_nrt_profile.argtypes = [ctypes.c_char_p]
    lib.axon_stop_nrt_profile.restype = ctypes.c_int64

    @contextlib.contextmanager
    def _hook(output_dir, device_ids):
        import jax

        jax.devices()
        if device_ids:
            ids = (ctypes.c_int64 * len(device_ids))(*device_ids)
            rc = lib.axon_start_nrt_profile(ids, len(device_ids))
        else:
            rc = lib.axon_start_nrt_profile(None, 0)
        if rc != 0:
            raise RuntimeError(f"axon_start_nrt_profile rc={rc}")
        try:
            yield
        finally:
            n = lib.axon_stop_nrt_profile(str(output_dir).encode())
            print(f"profile: {n} file(s) written to {output_dir}", file=sys.stderr)

    mod = types.ModuleType("antenv.axon_hooks")
    holder = [_hook]
    mod.get_axon_ntff_profile_hook = lambda: holder[0]
    mod.set_axon_ntff_profile_hook = lambda h: holder.__setitem__(0, h)
    sys.modules["antenv.axon_hooks"] = mod
    antenv.axon_hooks = mod


_install_ntff_hook()

# ---------------------------------------------------------------------------
# Problem constants (hardcoded per the contract)
# ---------------------------------------------------------------------------
B, S, D = 4, 2048, 1024
H, DK = 16, 64
N_CORES = 8
HPC = 8            # heads per core
NPAIR = HPC // 2   # head pairs per core = 4
FC = HPC * DK      # features per core = 512
SCALE = 1.0 / float(np.sqrt(DK))  # 0.125

KC = D // 128      # 8 contraction chunks for qkv projections
NSC = 4            # seq chunks of 512 for x / QK tiles
KB = S // 128      # 16 key blocks
NQC = 4            # q chunks of 512
LAG = 4            # AV trails scores by this many kb steps

_CACHED = {}


def _build():
    import concourse.tile as tile
    from concourse import bacc, mybir

    f32 = mybir.dt.float32
    bf16 = mybir.dt.bfloat16
    Exp = mybir.ActivationFunctionType.Exp

    nc = bacc.Bacc("TRN2", target_bir_lowering=False, debug=False,
                   num_devices=N_CORES)

    # Pre-swizzled DRAM inputs (host packs these; contiguous per partition)
    xs = nc.dram_tensor("xs", [NSC, 128, KC, 512], bf16,
                        kind="ExternalInput").ap()
    wqk = nc.dram_tensor("wqk", [NPAIR, 128, 2, KC, 128], bf16,
                         kind="ExternalInput").ap()
    wv = nc.dram_tensor("wv", [128, KC, FC], bf16, kind="ExternalInput").ap()
    wp = nc.dram_tensor("wp", [128, NPAIR, D], bf16,
                        kind="ExternalInput").ap()
    out = nc.dram_tensor("out", [S, D], bf16, kind="ExternalOutput").ap()
    dbg_on = os.environ.get("BASS_DEBUG_DUMP", "0") == "1"
    if dbg_on:
        dqt = nc.dram_tensor("dqt", [128, S], bf16, kind="ExternalOutput").ap()
        dkt = nc.dram_tensor("dkt", [128, S], bf16, kind="ExternalOutput").ap()
        dvt = nc.dram_tensor("dvt", [128, KB, FC], bf16,
                             kind="ExternalOutput").ap()
        datn = nc.dram_tensor("datn", [128, NPAIR, S], bf16,
                              kind="ExternalOutput").ap()
        dacc = nc.dram_tensor("dacc", [128, 2, 512], bf16,
                              kind="ExternalOutput").ap()

    with tile.TileContext(nc) as tc:
        with (
            tc.tile_pool(name="xsp", bufs=NSC) as xs_pool,
            tc.tile_pool(name="wqkp", bufs=NPAIR) as wqk_pool,
            tc.tile_pool(name="wvp", bufs=1) as wv_pool,
            tc.tile_pool(name="wpp", bufs=1) as wp_pool,
            tc.tile_pool(name="qkp", bufs=2) as qk_pool,
            tc.tile_pool(name="vtp", bufs=1) as v_pool,
            tc.tile_pool(name="ptp", bufs=24) as pt_pool,
            tc.tile_pool(name="accp", bufs=2) as acc_pool,
            tc.tile_pool(name="invp", bufs=2) as inv_pool,
            tc.tile_pool(name="atnp", bufs=1) as atn_pool,
            tc.tile_pool(name="outp", bufs=3) as out_pool,
            tc.tile_pool(name="cstp", bufs=1) as cst_pool,
            tc.tile_pool(name="stp", bufs=2, space="PSUM") as st_pool,
            tc.tile_pool(name="avp", bufs=2, space="PSUM") as av_pool,
            tc.tile_pool(name="shp", bufs=2, space="PSUM") as sh_pool,
        ):
            # ---------------- static tiles ----------------
            ones = cst_pool.tile([128, 64], bf16, tag="ones")
            nc.gpsimd.memset(ones[:], 1.0)

            XS = [xs_pool.tile([128, KC, 512], bf16, tag="xs",
                               name=f"xs_{sc}") for sc in range(NSC)]
            WV = wv_pool.tile([128, KC, FC], bf16, tag="wv")
            WP = wp_pool.tile([128, NPAIR, D], bf16, tag="wp")
            Vt = v_pool.tile([128, KB, FC], bf16, tag="vt")
            ATN = atn_pool.tile([128, NPAIR, S], bf16, tag="atn")

            WQK = []
            for p in range(NPAIR):
                t = wqk_pool.tile([128, 2, KC, 128], bf16, tag="wqk",
                                  name=f"wqk_{p}")
                WQK.append(t)

            # DMA order: pair0 weights + x chunk 0 first (prologue critical
            # path), then everything else.
            nc.sync.dma_start(WQK[0][:], wqk[0])
            nc.sync.dma_start(XS[0][:], xs[0])
            nc.sync.dma_start(XS[1][:], xs[1])
            nc.sync.dma_start(WV[:], wv[:])
            for sc in range(2, NSC):
                nc.sync.dma_start(XS[sc][:], xs[sc])
            for p in range(1, NPAIR):
                nc.sync.dma_start(WQK[p][:], wqk[p])
            nc.sync.dma_start(WP[:], wp[:])

            # QT/KT tiles per pair (ring of 2)
            QK = {}

            def alloc_qk(p):
                qt = qk_pool.tile([128, S], bf16, tag="qt", name=f"qt_{p}")
                kt = qk_pool.tile([128, S], bf16, tag="kt", name=f"kt_{p}")
                QK[p] = (qt, kt)

            # ---------------- filler quanta ----------------
            uid = [0]

            def proj_quantum(dst_ap, w_ap, x_ap):
                """dst_ap [128,512]bf16 <- sum_kc w_ap[:,kc,:].T @ x_ap[:,kc,:]"""
                uid[0] += 1
                ps = sh_pool.tile([128, 512], f32, tag="sh",
                                  name=f"prj{uid[0]}")
                pslice = ps[:]
                first = None
                for kc in range(KC):
                    mm = nc.tensor.matmul(pslice, lhsT=w_ap[:, kc, :],
                                          rhs=x_ap[:, kc, :],
                                          start=(kc == 0),
                                          stop=(kc == KC - 1))
                    first = first or mm
                nc.vector.tensor_copy(dst_ap, pslice)
                return first

            def v_quantum(sc, sbl):
                kb = sc * 4 + sbl
                uid[0] += 1
                ps = sh_pool.tile([128, 512], f32, tag="sh", name=f"v{kb}")
                first = None
                for kc in range(KC):
                    mm = nc.tensor.matmul(
                        ps[:], lhsT=XS[sc][:, kc, sbl * 128:(sbl + 1) * 128],
                        rhs=WV[:, kc, :],
                        start=(kc == 0), stop=(kc == KC - 1))
                    first = first or mm
                nc.vector.tensor_copy(Vt[:, kb, :], ps[:])
                return first

            def qk_fillers(p):
                alloc_qk(p)
                qt, kt = QK[p]
                w = WQK[p]
                thunks = []
                for sc in range(NSC):
                    thunks.append(lambda sc=sc: proj_quantum(
                        kt[:, sc * 512:(sc + 1) * 512], w[:, 1], XS[sc]))
                for sc in range(NSC):
                    thunks.append(lambda sc=sc: proj_quantum(
                        qt[:, sc * 512:(sc + 1) * 512], w[:, 0], XS[sc]))
                return thunks

            def cproj_quantum(qc, qb, nn):
                ps = sh_pool.tile([128, 512], f32, tag="sh",
                                  name=f"cp{qc}_{qb}_{nn}")
                first = None
                for p in range(NPAIR):
                    mm = nc.tensor.matmul(
                        ps[:],
                        lhsT=ATN[:, p, qc * 512 + qb * 128:
                                 qc * 512 + (qb + 1) * 128],
                        rhs=WP[:, p, nn * 512:(nn + 1) * 512],
                        start=(p == 0), stop=(p == NPAIR - 1))
                    first = first or mm
                ot = out_pool.tile([128, 512], bf16, tag="ot",
                                   name=f"o{qc}_{qb}_{nn}")
                nc.vector.tensor_copy(ot[:], ps[:])
                r0 = qc * 512 + qb * 128
                nc.sync.dma_start(
                    out[r0:r0 + 128, nn * 512:(nn + 1) * 512], ot[:])
                return first

            def cproj_fillers(qc):
                return [lambda qb=qb, nn=nn: cproj_quantum(qc, qb, nn)
                        for qb in range(4) for nn in range(2)]

            # ---------------- prologue ----------------
            # KT pair0 fully + QT pair0 chunk 0; alternate the psum target
            # between the shared bank and an st-ring slot so the chains
            # double-buffer against their DVE casts. V rides as the first
            # block's fillers (one V block per kb step, AV lags by LAG).
            alloc_qk(0)
            qt0, kt0 = QK[0]
            chains = [
                (kt0[:, 0:512], WQK[0][:, 1], XS[0]),
                (qt0[:, 0:512], WQK[0][:, 0], XS[0]),
                (qt0[:, 512:1024], WQK[0][:, 0], XS[1]),
            ]
            for dst, w, x in chains:
                proj_quantum(dst, w, x)

            # ---------------- per-pair filler queues ----------------
            # Emission deadlines (Tile deps follow emission order!):
            #   V(kb)   before av_step(kb) of block (p0,qc0)  -> slot kb
            #   qt0[sc] before scores of block (p0,qc=sc)
            #   QK(p+1) fully before pair p+1's first block
            #   cproj(qc) only after boundary(3,qc) was emitted
            late_kt0 = [
                lambda sc=sc: proj_quantum(
                    kt0[:, sc * 512:(sc + 1) * 512], WQK[0][:, 1], XS[sc])
                for sc in range(1, NSC)]
            late_qt0 = [
                lambda sc=sc: proj_quantum(
                    qt0[:, sc * 512:(sc + 1) * 512], WQK[0][:, 0], XS[sc])
                for sc in range(2, NSC)]
            v_all = [lambda sc=sc, sbl=sbl: v_quantum(sc, sbl)
                     for sc in range(NSC) for sbl in range(4)]
            # Queue order = emission deadlines: KT chunk j before scores
            # kb=4j; V(kb) at slot kb+3 (av pops at kb+4); remaining QT
            # chunks before their q-chunk's block; then pair-1's QK.
            filler_q = {
                0: late_kt0 + v_all + late_qt0 + qk_fillers(1),
                1: qk_fillers(2),
                2: qk_fillers(3),
                3: [],
            }
            # filler cadence per pair: p0 every step (V deadline), p1/p2
            # sparse (spread PE load), p3 every other step (8 cproj quanta)
            cadence = {0: 1, 1: 4, 2: 4, 3: 2}

            # ---------------- boundary ----------------
            def emit_boundary(pend):
                p, qc, avs, accA = pend
                dpsb = sh_pool.tile([128, 512], f32, tag="sh",
                                    name=f"dps_{p}_{qc}")
                for h in range(2):
                    osl = dpsb[h * 64:(h + 1) * 64, :]
                    nc.tensor.matmul(osl, lhsT=ones[:], rhs=accA[:, h, :],
                                     start=True, stop=True,
                                     tile_position=(0, h * 64),
                                     skip_group_check=True)
                inv = inv_pool.tile([128, 512], f32, tag="inv",
                                    name=f"inv_{p}_{qc}")
                nc.vector.reciprocal_approx_fast(inv[:], dpsb[:])
                nc.vector.tensor_mul(
                    ATN[:, p, qc * 512:(qc + 1) * 512], avs[:], inv[:])

            # ---------------- main attention blocks ----------------
            # Scores/exp form one continuous high-priority stream; AV steps
            # trail globally (across block boundaries) via a deque so late V
            # blocks or boundary work can never stall the exp stream. Each
            # block's boundary is emitted right after its last AV.
            av_pending = []  # (av_fn, boundary_tuple_or_None)

            def pump_avs(min_keep):
                while len(av_pending) > min_keep:
                    av_fn, bnd = av_pending.pop(0)
                    av_fn()
                    if bnd is not None:
                        emit_boundary(bnd)
                        if bnd[0] == 3 and bnd[1] < NQC - 1:
                            filler_q[3].extend(cproj_fillers(bnd[1]))

            gstep = [0]
            last_exp = [None]

            def _dep(inst):
                return getattr(inst, "ins", inst)
            for p in range(NPAIR):
                QTp, KTp = QK[p]
                fq = filler_q[p]
                fi = [0]

                def filler_step(fq=fq, fi=fi):
                    if fi[0] < len(fq):
                        # Scheduling-only dependency: the quantum's first MM
                        # may not be scheduled before the latest exp, so the
                        # frozen per-engine order interleaves chains with the
                        # exp stream instead of front-running it.
                        first = fq[fi[0]]()
                        if first is not None and last_exp[0] is not None:
                            tile.add_dep_helper(
                                _dep(first), _dep(last_exp[0]),
                                reason="filler paced by exp stream")
                        fi[0] += 1

                for qc in range(NQC):
                    qsl = slice(qc * 512, (qc + 1) * 512)
                    avs = av_pool.tile([128, 512], f32, tag="avs",
                                       name=f"avs_{p}_{qc}")
                    accA = acc_pool.tile([128, 2, 512], bf16, tag="accA",
                                         name=f"accA_{p}_{qc}")
                    pts = {}

                    def scores_step(kb, p=p, qc=qc, QTp=QTp, KTp=KTp,
                                    accA=accA, pts=pts, qsl=qsl):
                        st = st_pool.tile([128, 2, 512], f32, tag="st",
                                          name=f"st_{p}_{qc}_{kb}")
                        ksl = slice(kb * 128, (kb + 1) * 128)
                        pt = pt_pool.tile([128, 2, 512], bf16, tag="pt",
                                          name=f"pt_{p}_{qc}_{kb}")
                        with tc.high_priority(offset=100000):
                            for h in range(2):
                                hsl = slice(64 * h, 64 * h + 64)
                                nc.tensor.matmul(st[:, h, :],
                                                 lhsT=KTp[hsl, ksl],
                                                 rhs=QTp[hsl, qsl],
                                                 start=True, stop=True)
                            last_exp[0] = nc.scalar.activation(
                                pt[:], st[:], Exp, scale=SCALE)
                        pts[kb] = pt
                        if kb == 0:
                            nc.vector.tensor_copy(accA[:], pt[:])
                        else:
                            nc.vector.tensor_add(accA[:], accA[:], pt[:])

                    def av_step(kb, p=p, avs=avs, pts=pts):
                        pt = pts.pop(kb)
                        for h in range(2):
                            nc.tensor.matmul(
                                avs[64 * h:64 * h + 64, :],
                                lhsT=Vt[:, kb, p * 128 + 64 * h:
                                        p * 128 + 64 * h + 64],
                                rhs=pt[:, h, :],
                                start=(kb == 0), stop=(kb == KB - 1),
                                tile_position=(0, 64 * h),
                                skip_group_check=True)

                    cad = cadence[p]
                    for kb in range(KB):
                        scores_step(kb)
                        gstep[0] += 1
                        bnd = (p, qc, avs, accA) if kb == KB - 1 else None
                        av_pending.append(
                            (lambda kb=kb, f=av_step: f(kb), bnd))
                        pump_avs(LAG)
                        if kb % cad == 0:
                            filler_step()
                    filler_step()

                # drain remaining fillers before the next pair needs them
                while fi[0] < len(fq):
                    fq[fi[0]]()
                    fi[0] += 1

            # tail: drain AVs (emits the last boundary) + final c_proj
            lastacc = av_pending[-1][1][3]
            pump_avs(0)
            for q in cproj_fillers(NQC - 1):
                q()
            if dbg_on:
                qtd, ktd = QK[0]
                nc.sync.dma_start(dqt[:, :], qtd[:, :])
                nc.sync.dma_start(dkt[:, :], ktd[:, :])
                nc.sync.dma_start(dvt[:, :, :], Vt[:, :, :])
                nc.sync.dma_start(datn[:, :, :], ATN[:, :, :])
                nc.sync.dma_start(dacc[:, :, :], lastacc[:, :, :])

    nc.compile()
    return nc


def _get_nc():
    if "nc" not in _CACHED:
        _CACHED["nc"] = _build()
    return _CACHED["nc"]


def _shard(x, W_attn, W_proj):
    """Build per-core input maps with pre-swizzled bf16 layouts."""
    bf = ml_dtypes.bfloat16
    x = np.asarray(x, dtype=np.float32)
    W_attn = np.asarray(W_attn, dtype=np.float32)
    W_proj = np.asarray(W_proj, dtype=np.float32)
    in_maps = []
    for c in range(N_CORES):
        b, g = c // 2, c % 2
        # xs[sc, part, kc, j] = x[b, sc*512+j, kc*128+part]
        xt = x[b].T                                  # [D, S]
        xs_ = xt.reshape(KC, 128, NSC, 512).transpose(2, 1, 0, 3)
        # wqk[p, part, t, kc, f] = W_attn[kc*128+part, t*D + g*FC + p*128+f]
        # (partition dim second so the [128,2,KC,128] SBUF tile DMA is a
        # straight linear copy)
        wqk_ = np.empty((NPAIR, 128, 2, KC, 128), dtype=bf)
        for t in range(2):
            wslab = W_attn[:, t * D + g * FC: t * D + (g + 1) * FC]  # [D,FC]
            wr = wslab.reshape(KC, 128, NPAIR, 128).transpose(2, 1, 0, 3)
            wqk_[:, :, t] = wr.astype(bf)
        wv_ = W_attn[:, 2 * D + g * FC: 2 * D + (g + 1) * FC]        # [D,FC]
        wv_ = wv_.reshape(KC, 128, FC).transpose(1, 0, 2).astype(bf)
        # wp[part, p, m] = W_proj[g*FC + p*128 + part, m]
        wp_ = W_proj[g * FC:(g + 1) * FC, :].reshape(NPAIR, 128, D)
        wp_ = wp_.transpose(1, 0, 2).astype(bf)
        in_maps.append({
            "xs": np.ascontiguousarray(xs_.astype(bf)),
            "wqk": np.ascontiguousarray(wqk_),
            "wv": np.ascontiguousarray(wv_),
            "wp": np.ascontiguousarray(wp_),
        })
    return in_maps


def kernel(x, W_attn, W_proj):
    from concourse.bass_utils import run_bass_kernel_spmd

    nc = _get_nc()
    in_maps = _shard(x, W_attn, W_proj)
    trace = os.environ.get("BASS_PROBLEM_TRACE", "0") == "1"
    res = run_bass_kernel_spmd(nc, in_maps, list(range(N_CORES)), trace=trace)
    _CACHED["last_result"] = res
    out = np.empty((B, S, D), dtype=np.float32)
    for b in range(B):
        oa = np.asarray(res.results[2 * b]["out"])
        ob = np.asarray(res.results[2 * b + 1]["out"])
        if oa.dtype != np.float32:
            oa = oa.view(ml_dtypes.bfloat16).astype(np.float32)
            ob = ob.view(ml_dtypes.bfloat16).astype(np.float32)
        out[b] = oa + ob
    return out
